# revision 2
# baseline (speedup 1.0000x reference)
"""nn_CausalSelfAttention (B=4, T=2048, C=768, H=12) on 8 Trainium2 cores, v2.

Same sharding as v1 (core c -> units [6c, 6c+6), 3 half-blocks A/B/C =
q/k/v sources), but restructured for speed:

- Phase 1 computes the q/k source blocks TRANSPOSED (W^T @ x^T -> [feat, tok]
  in SBUF), so window extraction needs no PE transposes and no DRAM bounce
  for q/k. Bias is folded in via a K=1 ones-outer-product matmul.
- Window extraction = 144 strided DVE copies ([64, ~171] blocks, dst stride
  12) out of qkT; a partition-swapped duplicate (qkTs, built with 2 DMAs per
  block-half) keeps every copy same-partition-base.
- v block stays row-major, bounced through DRAM and gathered flat (affine
  only on flat addresses).
- QK matmuls for a head pair (partitions 0-63 / 64-127) alternate PE row
  groups via auto tile_position -> concurrent on HW. exp runs on [128, 1024]
  PSUM groups (A|B x 2 k-tiles) -> fewer ACT instructions.
- Program order pipelines: ph1(A0,B0) -> extract pair 0 -> attention pair 0
  overlaps remaining ph1; ph3 tiles interleave with pair-2 attention.
"""

import sys

for _p in ("/opt/trn_rl_repo",):
    if _p not in sys.path:
        sys.path.insert(0, _p)

import numpy as np

import concourse.bacc as bacc
import concourse.bass as bass
import concourse.mybir as mybir
import concourse.tile as tile
from concourse import library_config

B, T, C, H, HD = 4, 2048, 768, 12, 64
HL = H // 2          # 6 units per core
FQ = HL * HD         # 384
QC = 256             # query chunk
KP = 128             # key tile
N_CORES = 8

FP32 = mybir.dt.float32
BF16 = mybir.dt.bfloat16
AX = mybir.AluOpType
EXP = mybir.ActivationFunctionType.Exp


def build_nc(t=T, debug=False, dump=False, reps=1, phases="all"):
    assert t % 512 == 0
    n_qc = t // QC
    n_tt = t // KP
    n_cc = C // 128          # 6
    th = t // 2
    hb = th * C
    wsz = 64 * t             # flat elems per window
    Lmax = -(-t // 12)       # 171 for t=2048
    wpad = 12 * Lmax         # 2052
    NCH = min(512, th)
    n_nch = th // NCH
    vblk = n_tt * (HD + 1)   # vt elems per window

    nc = bacc.Bacc("TRN2", target_bir_lowering=False, debug=debug,
                   num_devices=N_CORES)

    xT3_d = nc.dram_tensor("xT3", [C, 3 * th], BF16, kind="ExternalInput")
    w3_d = nc.dram_tensor("w3", [C, 3 * C], BF16, kind="ExternalInput")
    b3_d = nc.dram_tensor("b3", [1, 3 * C], BF16, kind="ExternalInput")
    wp_d = nc.dram_tensor("wp", [FQ, C], BF16, kind="ExternalInput")
    out_d = nc.dram_tensor("out", [t, C], FP32, kind="ExternalOutput")
    if dump:
        qkT_dump = nc.dram_tensor("qkT_dump", [128, 2 * n_cc * th], BF16,
                                  kind="ExternalOutput")
        qT_dump = nc.dram_tensor("qT_dump", [128, 3 * wpad], BF16,
                                 kind="ExternalOutput")
        kT_dump = nc.dram_tensor("kT_dump", [128, 3 * wpad], BF16,
                                 kind="ExternalOutput")
        vt_dump = nc.dram_tensor("vt_dump", [128, HL * vblk], BF16,
                                 kind="ExternalOutput")
        yT_dump = nc.dram_tensor("yT_dump", [128, 3 * t], BF16,
                                 kind="ExternalOutput")

    with tile.TileContext(nc) as tc:
        nc.gpsimd.load_library(library_config.attn)
        with (
            tc.tile_pool(name="const", bufs=1) as cp,
            tc.tile_pool(name="dramp", bufs=1, space="DRAM") as dp,
        ):
            blk_v = dp.tile([hb], BF16, tag="blkv")
            xT3 = cp.tile([128, n_cc * 3 * th], BF16, tag="xT3")
            w3 = cp.tile([128, n_cc * 3 * C], BF16, tag="w3")
            b3r = cp.tile([1, 3 * C], BF16, tag="b3r")
            ones = cp.tile([1, NCH], BF16, tag="ones")
            wp = cp.tile([128, 3 * C], BF16, tag="wp")
            qkT = cp.tile([128, 2 * n_cc * th], BF16, tag="qkT")
            qkTs = cp.tile([128, 2 * n_cc * th], BF16, tag="qkTs")
            qT = cp.tile([128, 3 * wpad], BF16, tag="qT")
            kT = cp.tile([128, 3 * wpad], BF16, tag="kT")
            vt = cp.tile([128, HL * vblk], BF16, tag="vt")
            yT = cp.tile([128, 3 * t], BF16, tag="yT")

            tb = 3 * th
            nc.sync.dma_start(out=b3r[:], in_=b3_d[:])
            nc.vector.memset(ones[:], 1.0)
            # x loads ride the (idle-at-start) ACT HWDGE queue so weight and
            # activation input streams transfer in parallel.
            for kc in range(n_cc):
                nc.sync.dma_start(out=w3[:, 3 * C * kc:3 * C * (kc + 1)],
                                  in_=w3_d[128 * kc:128 * (kc + 1), :])
                nc.scalar.dma_start(
                    out=xT3[:, tb * kc:tb * kc + th],
                    in_=xT3_d[128 * kc:128 * (kc + 1), 0:th])
            for j in (1, 2):
                for kc in range(n_cc):
                    nc.scalar.dma_start(
                        out=xT3[:, tb * kc + th * j:tb * kc + th * (j + 1)],
                        in_=xT3_d[128 * kc:128 * (kc + 1), th * j:th * (j + 1)])
            for pc in range(3):
                nc.sync.dma_start(out=wp[:, C * pc:C * (pc + 1)],
                                  in_=wp_d[128 * pc:128 * (pc + 1), :])

            def xT3_c(kc, j):
                base = tb * kc + th * j
                return xT3[:, base:base + th]

            def vt_u(wl):
                return vt[:, vblk * wl:vblk * (wl + 1)]

            # ---------------- phase/loop bodies -----------------------
            def emit_ab(ab_pool, b, nch):
                """Transposed projection for block b in {0,1}: qkT[feat, tok]
                token chunk nch."""
                for ft in range(n_cc):
                    ps = ab_pool.tile([128, 512], FP32, tag="pps")
                    for kc in range(n_cc):
                        nc.tensor.matmul(
                            ps[:, :NCH],
                            lhsT=w3[:, 3 * C * kc + C * b + 128 * ft:
                                    3 * C * kc + C * b + 128 * (ft + 1)],
                            rhs=xT3_c(kc, b)[:, NCH * nch:NCH * (nch + 1)],
                            start=(kc == 0), stop=False)
                    nc.tensor.matmul(
                        ps[:, :NCH],
                        lhsT=b3r[0:1, C * b + 128 * ft:C * b + 128 * (ft + 1)],
                        rhs=ones[0:1, :NCH],
                        start=False, stop=True)
                    nc.vector.tensor_copy(
                        qkT[:, n_cc * th * b + th * ft + NCH * nch:
                            n_cc * th * b + th * ft + NCH * (nch + 1)],
                        ps[:, :NCH])

            def emit_dup(nch):
                """Partition-swapped copy of qkT token chunk nch (both blocks)."""
                for b in (0, 1):
                    base = n_cc * th * b
                    sv = qkT[:, base:base + n_cc * th].rearrange(
                        "p (f c) -> p f c", c=th)[:, :, NCH * nch:NCH * (nch + 1)]
                    dv = qkTs[:, base:base + n_cc * th].rearrange(
                        "p (f c) -> p f c", c=th)[:, :, NCH * nch:NCH * (nch + 1)]
                    nc.sync.dma_start(out=dv[0:64], in_=sv[64:128])
                    nc.sync.dma_start(out=dv[64:128], in_=sv[0:64])

            def emit_c(c_pool, ob_pool, ti):
                """Row-major projection for the v block, token-tile ti -> DRAM."""
                for n0, n1 in ((0, 512), (512, C)):
                    w_ = n1 - n0
                    ps = c_pool.tile([128, 512], FP32, tag="pps")
                    for kc in range(n_cc):
                        nc.tensor.matmul(
                            ps[:, :w_],
                            lhsT=xT3_c(kc, 2)[:, 128 * ti:128 * (ti + 1)],
                            rhs=w3[:, 3 * C * kc + 2 * C + n0:
                                   3 * C * kc + 2 * C + n1],
                            start=(kc == 0), stop=False)
                    nc.tensor.matmul(
                        ps[:, :w_],
                        lhsT=ones[0:1, 0:128],
                        rhs=b3r[0:1, 2 * C + n0:2 * C + n1],
                        start=False, stop=True)
                    ob = ob_pool.tile([128, 512], BF16, tag="csb")
                    nc.vector.tensor_copy(ob[:, :w_], ps[:, :w_])
                    base = 128 * ti * C
                    nc.sync.dma_start(
                        out=blk_v[base:base + 128 * C].rearrange(
                            "(p f) -> p f", f=C)[:, n0:n1],
                        in_=ob[:, :w_])

            def emit_v(w):
                v3 = vt_u(w).rearrange("p (n e) -> p n e", e=HD + 1)
                nc.sync.dma_start(
                    out=v3[:, :, 0:HD],
                    in_=blk_v[wsz * w:wsz * (w + 1)].rearrange(
                        "(n p d) -> p n d", p=128, d=HD))
                nc.vector.memset(v3[:, :, HD:HD + 1], 1.0)

            def emit_ext(w, b, dT, eng=None):
                """Extract window w of block b into dT (qT or kT), t-order."""
                pi, po = w // 2, 64 * (w % 2)
                A0, B0 = (w * t) // 12, (w * t) % 12
                dv = dT[:, wpad * pi:wpad * (pi + 1)].rearrange(
                    "p (m r) -> p m r", r=12)
                eng = eng or nc.vector
                for r in range(12):
                    h = (B0 + r) % 12
                    rho = A0 + (B0 + r) // 12
                    L = (t - r + 11) // 12
                    sb = 64 * (h % 2)
                    src_t = qkT if sb == po else qkTs
                    base = n_cc * th * b + th * (h // 2) + rho
                    eng.tensor_copy(
                        dv[po:po + 64, 0:L, r:r + 1],
                        src_t[po:po + 64, base:base + L])

            def emit_attn(s_pool, p_pool, y_pool, n_pool, pi, j,
                          sub="all"):
                """Attention for head pair pi, query chunk j."""
                nkt = (QC * (j + 1)) // KP
                y_ps = None
                if sub != "qkexp":
                    y_ps = y_pool.tile([65, 2 * QC], FP32, tag="yps")
                qA = qT[0:64, wpad * pi + QC * j:wpad * pi + QC * (j + 1)]
                qB = qT[64:128, wpad * pi + QC * j:wpad * pi + QC * (j + 1)]
                GK = 2
                for g0 in range(0, nkt, GK):
                    gsz = min(GK, nkt - g0)
                    W = gsz * QC
                    s_ps = s_pool.tile([128, 2 * GK * QC], FP32, tag="sps")
                    p_sb = p_pool.tile([128, 2 * GK * QC], BF16, tag="psb")
                    do_pv = sub in ("all", "nonorm")
                    for tt in range(gsz):
                        kt = g0 + tt
                        ksl = slice(wpad * pi + KP * kt,
                                    wpad * pi + KP * (kt + 1))
                        nc.tensor.matmul(
                            s_ps[:, QC * tt:QC * (tt + 1)],
                            lhsT=kT[0:64, ksl], rhs=qA,
                            start=True, stop=True)
                        nc.tensor.matmul(
                            s_ps[:, W + QC * tt:W + QC * (tt + 1)],
                            lhsT=kT[64:128, ksl], rhs=qB,
                            start=True, stop=True)
                    nc.scalar.activation(p_sb[:, :2 * W], s_ps[:, :2 * W], EXP)
                    for tt in range(gsz):
                        kt = g0 + tt
                        d = kt - (QC * j) // KP
                        if d >= 0:
                            mv = p_sb[:, :2 * W].rearrange(
                                "p (h x) -> p h x", h=2)[:, :,
                                                         QC * tt:QC * (tt + 1)]
                            nc.gpsimd.affine_select(
                                out=mv, in_=mv,
                                pattern=[[0, 2], [1, QC]],
                                channel_multiplier=-1,
                                base=-KP * d,
                                compare_op=AX.is_ge,
                                fill=0.0)
                    if not do_pv:
                        continue
                    # one accumulation group for the whole y bank: start only
                    # on the very first MM (zero-region covers both heads'
                    # cols), stop on the very last.
                    for tt in range(gsz):
                        kt = g0 + tt
                        nc.tensor.matmul(
                            y_ps[:, 0:QC],
                            lhsT=vt_u(2 * pi)[:, 65 * kt:65 * (kt + 1)],
                            rhs=p_sb[:, QC * tt:QC * (tt + 1)],
                            start=(kt == 0), stop=False)
                        nc.tensor.matmul(
                            y_ps[:, QC:2 * QC],
                            lhsT=vt_u(2 * pi + 1)[:, 65 * kt:65 * (kt + 1)],
                            rhs=p_sb[:, W + QC * tt:W + QC * (tt + 1)],
                            start=False, stop=(kt == nkt - 1))
                if sub != "all":
                    return
                # normalize; A writes yT directly, B stages + DMA partition
                # shift. One reciprocal + one broadcast covers both heads.
                rd = n_pool.tile([1, 2 * QC], FP32, tag="rd")
                nc.vector.reciprocal(rd[0:1, :], y_ps[64:65, :])
                bc = n_pool.tile([64, 2 * QC], FP32, tag="bc")
                nc.gpsimd.partition_broadcast(bc[:], rd[0:1, :])
                nc.vector.tensor_tensor(
                    out=yT[0:64, t * pi + QC * j:t * pi + QC * (j + 1)],
                    in0=y_ps[0:64, 0:QC], in1=bc[:, 0:QC], op=AX.mult)
                yb = n_pool.tile([64, QC], BF16, tag="yb")
                nc.vector.tensor_tensor(
                    out=yb[:], in0=y_ps[0:64, QC:2 * QC], in1=bc[:, QC:2 * QC],
                    op=AX.mult)
                nc.sync.dma_start(
                    out=yT[64:128, t * pi + QC * j:t * pi + QC * (j + 1)],
                    in_=yb[:])

            def emit_ph3(o_pool, ob_pool, ti):
                for n0, n1 in ((0, 512), (512, C)):
                    w_ = n1 - n0
                    ps = o_pool.tile([128, 512], FP32, tag="ops")
                    for pc in range(3):
                        nc.tensor.matmul(
                            ps[:, :w_],
                            lhsT=yT[:, t * pc + 128 * ti:t * pc + 128 * (ti + 1)],
                            rhs=wp[:, C * pc + n0:C * pc + n1],
                            start=(pc == 0), stop=(pc == 2))
                    ob = ob_pool.tile([128, 512], FP32, tag="osb")
                    nc.vector.tensor_copy(ob[:, :w_], ps[:, :w_])
                    nc.sync.dma_start(
                        out=out_d[128 * ti:128 * (ti + 1), n0:n1],
                        in_=ob[:, :w_])

            # ---------------- program ---------------------------------
            # token-row coverage: pair p windows [2p, 2p+2) need qkT rows
            # [2p*Lmax, (2p+2)*Lmax) and blk_v rows likewise.
            n_ti = th // 128

            def ti_hi(p):
                # first C row-tile index strictly beyond pair p's windows
                return min(n_ti, -(-((2 * p + 2) * wsz // C) // 128))

            def nch_hi(p):
                return min(n_nch, -(-((2 * p + 2) * wsz // C) // NCH))

            for _rep in range(reps):
                with (
                    tc.tile_pool(name="sps", bufs=2, space="PSUM") as s_pool,
                    tc.tile_pool(name="yps", bufs=2, space="PSUM") as y_pool,
                    tc.tile_pool(name="psb", bufs=3) as p_pool,
                    tc.tile_pool(name="nrm", bufs=3) as n_pool,
                    tc.tile_pool(name="csb", bufs=3) as ob_pool,
                ):
                    do_prep = phases in ("all", "noattn")
                    do_attn = phases in ("all", "attnonly", "attn_qkexp",
                                         "attn_nonorm")
                    attn_sub = {"attn_qkexp": "qkexp",
                                "attn_nonorm": "nonorm"}.get(phases, "all")
                    with tc.tile_pool(name="ph1ps", bufs=2,
                                      space="PSUM") as ph1_pool:
                        # Pair-0 prerequisites first (attention p0 unblocks
                        # early); attention p0 outranks the remaining ph1 /
                        # extraction work, which fills PE/DVE slack while ACT
                        # chews pair 0, so pair 1 starts with no gap.
                        if phases.startswith("attn") and _rep == 0:
                            nc.vector.memset(qT[:], 0.01)
                            nc.vector.memset(kT[:], 0.01)
                            nc.vector.memset(vt[:], 0.01)
                            if phases != "attnonly":
                                nc.vector.memset(yT[:], 0.01)
                        if phases == "noattn" and _rep == 0:
                            nc.vector.memset(yT[:], 0.01)
                        # NCH=512 rows = exactly 3 windows: chunk 0 covers
                        # windows 0-2, chunk 1 covers 3-5 (t=2048).
                        wlo = min(6, (NCH * nch_hi(0) * C) // wsz)
                        if do_prep:
                            for nch in range(nch_hi(0)):
                                emit_ab(ph1_pool, 0, nch)
                                emit_ab(ph1_pool, 1, nch)
                                emit_dup(nch)
                            for ti in range(ti_hi(0)):
                                emit_c(ph1_pool, ob_pool, ti)
                            for w in range(min(wlo, 6)):
                                emit_ext(w, 0, qT)
                                emit_ext(w, 1, kT, nc.gpsimd)
                                if w < 2:
                                    emit_v(w)
                        if do_attn:
                            for j in range(n_qc):
                                emit_attn(s_pool, p_pool, y_pool, n_pool, 0, j, attn_sub)
                        if do_prep:
                            for nch in range(nch_hi(0), n_nch):
                                emit_ab(ph1_pool, 0, nch)
                                emit_ab(ph1_pool, 1, nch)
                                emit_dup(nch)
                            for w in range(min(wlo, 6), 6):
                                emit_ext(w, 0, qT)
                                emit_ext(w, 1, kT, nc.gpsimd)
                            for ti in range(ti_hi(0), n_ti):
                                emit_c(ph1_pool, ob_pool, ti)
                            for w in range(2, 6):
                                emit_v(w)
                        if do_attn:
                            for j in range(n_qc):
                                emit_attn(s_pool, p_pool, y_pool, n_pool, 1, j, attn_sub)
                    if dump:
                        nc.sync.dma_start(out=qkT_dump[:], in_=qkT[:])
                        nc.sync.dma_start(out=qT_dump[:], in_=qT[:])
                        nc.sync.dma_start(out=kT_dump[:], in_=kT[:])
                        nc.sync.dma_start(out=vt_dump[:], in_=vt[:])
                    # pair 2 attention interleaved with ph3
                    with tc.tile_pool(name="ops", bufs=2, space="PSUM") as o_pool:
                        for j in range(n_qc):
                            if do_attn:
                                emit_attn(s_pool, p_pool, y_pool, n_pool, 2, j, attn_sub)
                            for ti in range(2 * j, 2 * (j + 1)):
                                emit_ph3(o_pool, ob_pool, ti)
                    if dump:
                        nc.sync.dma_start(out=yT_dump[:], in_=yT[:])

    nc.compile()
    return nc


def shard_inputs(x, W_attn, b_attn, W_proj, t=T):
    """Host-side shard + cast. Returns in_maps (one dict per core)."""
    scale = np.float32(1.0 / np.sqrt(HD))
    bf16 = np.dtype(mybir.dt.np(BF16))
    Wpr = W_proj.reshape(H, HD, C)
    in_maps = []
    for c in range(N_CORES):
        c2, half = c // 2, c % 2
        rows = slice((t // 2) * half, (t // 2) * (half + 1))
        xs, ws, bs = [], [], []
        for j, u in enumerate([c2, 4 + c2, 8 + c2]):
            b_j, s_j = divmod(u, 3)
            sc = scale if j == 0 else np.float32(1.0)
            xs.append(np.ascontiguousarray(x[b_j, rows].T))
            ws.append(W_attn[:, C * s_j:C * (s_j + 1)] * sc)
            bs.append(b_attn[C * s_j:C * (s_j + 1)] * sc)
        xT3 = np.concatenate(xs, axis=1).astype(bf16)          # [C, 3*t/2]
        w3 = np.concatenate(ws, axis=1).astype(bf16)           # [C, 3C]
        b3 = np.concatenate(bs).reshape(1, 3 * C).astype(bf16)
        hs = slice(HL * half, HL * (half + 1))
        wpm = np.ascontiguousarray(Wpr[hs].reshape(FQ, C)).astype(bf16)
        in_maps.append({"xT3": np.ascontiguousarray(xT3), "w3": w3,
                        "b3": b3, "wp": wpm})
    return in_maps


LAST_RESULTS = None


def kernel(x, W_attn, b_attn, W_proj, b_proj):
    global LAST_RESULTS
    from concourse.bass_utils import run_bass_kernel_spmd

    x = np.asarray(x, dtype=np.float32)
    W_attn = np.asarray(W_attn, dtype=np.float32)
    b_attn = np.asarray(b_attn, dtype=np.float32)
    W_proj = np.asarray(W_proj, dtype=np.float32)
    b_proj = np.asarray(b_proj, dtype=np.float32)

    nc = build_nc()
    in_maps = shard_inputs(x, W_attn, b_attn, W_proj)
    res = run_bass_kernel_spmd(nc, in_maps, list(range(N_CORES)))
    LAST_RESULTS = res

    out = np.empty((B, T, C), dtype=np.float32)
    for b in range(B):
        out[b] = res.results[2 * b]["out"] + res.results[2 * b + 1]["out"] \
            + b_proj[None, :]
    return out
